# revision 13
# baseline (speedup 1.0000x reference)
"""ConditionalBlock (AdaLN transformer block) on 8 trn2 NeuronCores.

Sharding: core (b, p) for b in 0..3, p in 0..1 handles batch b, output T-blocks
{2i+p : i in 0..3}. Host permutes rows (pair-swap for odd p) so every core runs
the identical instruction stream (one NEFF, SPMD, no collectives); causal masks
are per-core data. All on-chip compute in feature-on-partition (transposed)
layout; matmuls in bf16 with fp32 PSUM accumulation.
"""
import os
import numpy as np
import ml_dtypes

import concourse.bass as bass
import concourse.mybir as mybir
import concourse.tile as tile
from concourse.vector_clock import ScopedClock
from concourse.bass_utils import run_bass_kernel_spmd

BF16 = mybir.dt.bfloat16
F32 = mybir.dt.float32
AF = mybir.ActivationFunctionType

D = 1024
T = 1024
H = 16
HD = 4096
NB = 8          # 128-blocks along T
NSLOT = 4       # q slots per core
EPS = 1e-5
ATT_SCALE = 64.0 ** -0.5
EXP_BIAS = -20.0

# ---------------------------------------------------------------- drain patch
_MAXW = 1


def _patched_drain_and_barrier(self, tick_clock, wait_clock):
    drain_inst = self.nc.sync.drain()
    wait_clock.add_sem_waits(
        drain_inst.ins, ScopedClock({None: tick_clock.global_clock})
    )
    si = drain_inst.ins.sync_info
    waits = list(si.on_wait) if si is not None else []
    if len(waits) > _MAXW:
        drain_inst.ins.sync_info = mybir.SyncInfo(
            on_wait=waits[:_MAXW], on_update=list(si.on_update)
        )
        for i in range(_MAXW, len(waits), _MAXW):
            nop = self.nc.sync.nop(nofuse=True, hint=f"drain_wait_{i}")
            nop.ins.sync_info = mybir.SyncInfo(
                on_wait=waits[i : i + _MAXW], on_update=[]
            )
    self.nc.all_engine_barrier()
    assert self.sems is not None
    popped = self.nc._tile_sem_poison_stack.pop()
    assert popped is self._sem_poison
    self.nc.clear_and_free_semaphores(list(self.sems.allocated().values()))
    self.nc.all_engine_barrier()


tile.TileContext._drain_and_barrier = _patched_drain_and_barrier


def even_cols(ap_2d):
    """[128, 1024] -> [128, 4, 128] selecting 128-col blocks 0,2,4,6."""
    return ap_2d.rearrange("p (s two c) -> p s two c", two=2, c=128)[:, :, 0, :]



_WAIT_CAP = 1


def _split_excess_waits(nc, cap=_WAIT_CAP):
    """Walrus codegen allows only a couple of sync-wait commands per
    instruction; move excess waits onto NOPs inserted just before."""
    n_split = 0
    for fn in nc.m.functions:
        for blk in fn.blocks:
            newlist = []
            for inst in blk.instructions:
                si = getattr(inst, "sync_info", None)
                if si is not None and si.on_wait and len(si.on_wait) > cap:
                    waits = list(si.on_wait)
                    for i in range(0, len(waits) - cap, cap):
                        n_split += 1
                        newlist.append(mybir.InstNoOp(
                            name=f"{inst.name}_w{i}",
                            engine=inst.engine,
                            bass_nofuse=True,
                            sync_info=mybir.SyncInfo(
                                on_wait=waits[i : i + cap], on_update=[]),
                        ))
                    inst.sync_info = mybir.SyncInfo(
                        on_wait=waits[len(waits) - cap :],
                        on_update=list(si.on_update))
                newlist.append(inst)
            blk.instructions[:] = newlist
    return n_split


def build_nc(repeat=1):
    nc = bass.Bass("TRN2")
    ext_in = dict(kind="ExternalInput")
    xT = nc.dram_tensor("xT", [D, T], BF16, **ext_in)
    wqkv = nc.dram_tensor("wqkv", [D, 3 * D], BF16, **ext_in)
    wproj = nc.dram_tensor("wproj", [D, D], BF16, **ext_in)
    w1r = nc.dram_tensor("w1r", [32, 128, 1024], BF16, **ext_in)
    w2r = nc.dram_tensor("w2r", [8, 128, 4096], BF16, **ext_in)
    modT_d = nc.dram_tensor("modT_d", [128, 48], F32, **ext_in)
    g1T = nc.dram_tensor("g1T", [128, 8], F32, **ext_in)
    be1T = nc.dram_tensor("be1T", [128, 8], F32, **ext_in)
    g2T = nc.dram_tensor("g2T", [128, 8], F32, **ext_in)
    be2T = nc.dram_tensor("be2T", [128, 8], F32, **ext_in)
    bprojT = nc.dram_tensor("bprojT", [128, 8], F32, **ext_in)
    b2T = nc.dram_tensor("b2T", [128, 8], F32, **ext_in)
    b1T = nc.dram_tensor("b1T", [128, 32], F32, **ext_in)
    masks = nc.dram_tensor("masks", [NSLOT, 2, 128, 128], BF16, **ext_in)
    ident = nc.dram_tensor("ident", [128, 128], BF16, **ext_in)
    outT = nc.dram_tensor("outT", [D, 512], F32, kind="ExternalOutput")

    with tile.TileContext(nc) as tc:
        with (
            tc.tile_pool(name="singles", bufs=1) as singles,
            tc.tile_pool(name="persist", bufs=1) as persist,
            tc.tile_pool(name="wrow", bufs=1) as wrow,
            tc.tile_pool(name="wstream", bufs=3) as wstream,
            tc.tile_pool(name="w2s", bufs=2) as w2s,
            tc.tile_pool(name="tmp", bufs=1) as tmp,
            tc.tile_pool(name="epool", bufs=6) as epool,
            tc.tile_pool(name="ps_mm", bufs=2, space="PSUM") as ps_mm,
            tc.tile_pool(name="ps_s", bufs=2, space="PSUM") as ps_s,
            tc.tile_pool(name="ps_tr", bufs=2, space="PSUM") as ps_tr,
            tc.tile_pool(name="ps_o", bufs=2, space="PSUM") as ps_o,
        ):
            # ---- constants
            id_sb = singles.tile([128, 128], BF16, name="t1")
            nc.sync.dma_start(out=id_sb, in_=ident[:, :])
            mask_sb = [[singles.tile([128, 128], BF16, tag=f"mask{i}{s}", name="t2")
                        for s in range(2)] for i in range(NSLOT)]
            for i in range(NSLOT):
                for s in range(2):
                    nc.sync.dma_start(out=mask_sb[i][s], in_=masks[i, s])
            ones_col = singles.tile([128, 1], BF16, name="t3")
            nc.vector.memset(ones_col, 1.0)
            ones_row = singles.tile([1, 128], BF16, name="t4")
            nc.vector.memset(ones_row, 1.0)
            eps_sb = singles.tile([1, 1], F32, name="t5")
            nb20 = singles.tile([128, 1], F32, name="nb20")
            nc.vector.memset(nb20, EXP_BIAS)
            nc.vector.memset(eps_sb, EPS)
            small_in = {}
            for name, t_ in (("g1T", g1T), ("be1T", be1T), ("g2T", g2T),
                             ("be2T", be2T), ("bprojT", bprojT), ("b2T", b2T)):
                sb = singles.tile([128, 8], F32, tag=name, name="t8")
                nc.sync.dma_start(out=sb, in_=t_[:, :])
                small_in[name] = sb
            b1T_sb = singles.tile([128, 32], F32, name="t9")
            nc.sync.dma_start(out=b1T_sb, in_=b1T[:, :])

            dbg = os.environ.get("KDBG", "")
            def _dump(row, ap, width=512):
                dt_ = tmp.tile([128, 512], F32, tag="dbgd", name=f"dbg{row}x")
                nc.vector.tensor_copy(out=dt_[:, :width], in_=ap)
                nc.sync.dma_start(out=outT[128 * row : 128 * (row + 1), 0:width],
                                  in_=dt_[:, :width])

            for rep in range(repeat):
                # ---- mod = silu(cond) @ Wada + bada: computed on host
                # (replicated AdaLN modulation vectors), loaded directly
                modT = persist.tile([128, 48], F32, tag="modT", name="t11")
                nc.sync.dma_start(out=modT, in_=modT_d[:, :])
                # combined LN+AdaLN scale/bias  (sc cols already have +1 folded)
                scaleA = persist.tile([128, 8], F32, tag="scaleA", name="t14")
                biasA = persist.tile([128, 8], F32, tag="biasA", name="t15")
                scaleM = persist.tile([128, 8], F32, tag="scaleM", name="t16")
                biasM = persist.tile([128, 8], F32, tag="biasM", name="t17")
                gba = persist.tile([128, 8], F32, tag="gba", name="t18")
                gbm = persist.tile([128, 8], F32, tag="gbm", name="t19")
                t8 = tmp.tile([128, 8], F32, tag="t8", name="t20")
                nc.vector.tensor_mul(scaleA, small_in["g1T"], modT[:, 8:16])
                nc.vector.tensor_mul(t8, small_in["be1T"], modT[:, 8:16])
                nc.vector.tensor_add(biasA, t8, modT[:, 0:8])
                nc.vector.tensor_mul(scaleM, small_in["g2T"], modT[:, 32:40])
                t8b = tmp.tile([128, 8], F32, tag="t8b", name="t21")
                nc.vector.tensor_mul(t8b, small_in["be2T"], modT[:, 32:40])
                nc.vector.tensor_add(biasM, t8b, modT[:, 24:32])
                nc.vector.tensor_mul(gba, modT[:, 16:24], small_in["bprojT"])
                nc.vector.tensor_mul(gbm, modT[:, 40:48], small_in["b2T"])

                # ---- load xT, LN1 stats over full T
                xT_sb = [persist.tile([128, T], BF16, tag=f"xT{k}", name="t22")
                         for k in range(8)]
                for k in range(8):
                    nc.sync.dma_start(out=xT_sb[k], in_=xT[128 * k : 128 * (k + 1), :])
                sums = tmp.tile([1, 2048], F32, tag="sums", name="t23")  # [mu(1024) | sq(1024)]
                for half in range(2):
                    sl = slice(512 * half, 512 * (half + 1))
                    mu_ps = ps_mm.tile([128, 512], F32, tag="mm", name="t24")
                    for k in range(8):
                        nc.tensor.matmul(mu_ps[0:1, :], ones_col, xT_sb[k][:, sl],
                                         start=(k == 0), stop=(k == 7))
                    nc.vector.tensor_copy(out=sums[:, sl], in_=mu_ps[0:1, :])
                    sq_ps = ps_mm.tile([128, 512], F32, tag="mm", name="t25")
                    for k in range(8):
                        xsqt = tmp.tile([128, 512], BF16, tag=f"xsq{k % 2}", name="t26")
                        nc.vector.tensor_mul(xsqt, xT_sb[k][:, sl], xT_sb[k][:, sl])
                        nc.tensor.matmul(sq_ps[0:1, :], ones_col, xsqt,
                                         start=(k == 0), stop=(k == 7))
                    nc.vector.tensor_copy(
                        out=sums[:, slice(1024 + 512 * half, 1024 + 512 * (half + 1))],
                        in_=sq_ps[0:1, :])
                # stats rows: mean, var, rstd
                mu_row = tmp.tile([1, T], F32, tag="mu_row", name="t27")
                nc.vector.tensor_scalar_mul(mu_row, sums[:, 0:1024], 1.0 / D)
                msq_row = tmp.tile([1, T], F32, tag="msq_row", name="t28")
                nc.vector.tensor_scalar_mul(msq_row, sums[:, 1024:2048], 1.0 / D)
                mu2 = tmp.tile([1, T], F32, tag="sums", name="t29")
                nc.vector.tensor_mul(mu2, mu_row, mu_row)
                var_row = tmp.tile([1, T], F32, tag="rowB", name="t30")
                nc.vector.tensor_sub(var_row, msq_row, mu2)
                std_row = tmp.tile([1, T], F32, tag="msq_row", name="t31")
                nc.scalar.activation(out=std_row, in_=var_row, func=AF.Sqrt,
                                     bias=eps_sb, scale=1.0)
                r_row = tmp.tile([1, T], F32, tag="rowB2", name="t32")
                nc.vector.reciprocal(r_row, std_row)
                mu_b = tmp.tile([1, T], BF16, tag="mu_b", name="t33")
                nc.vector.tensor_copy(out=mu_b, in_=mu_row)
                r_b = tmp.tile([1, T], BF16, tag="r_b", name="t34")
                nc.vector.tensor_copy(out=r_b, in_=r_row)
                # broadcast to [128, T]
                mu_bc = persist.tile([128, T], BF16, tag="mu_bc", name="t35")
                r_bc = persist.tile([128, T], BF16, tag="r_bc", name="t36")
                for half in range(2):
                    sl = slice(512 * half, 512 * (half + 1))
                    pb = ps_mm.tile([128, 512], F32, tag="mm", name="t37")
                    nc.tensor.matmul(pb, ones_row, mu_b[:, sl], start=True, stop=True)
                    nc.vector.tensor_copy(out=mu_bc[:, sl], in_=pb)
                    pb2 = ps_mm.tile([128, 512], F32, tag="mm", name="t38")
                    nc.tensor.matmul(pb2, ones_row, r_b[:, sl], start=True, stop=True)
                    nc.vector.tensor_copy(out=r_bc[:, sl], in_=pb2)
                if dbg == "mubc":
                    _dump(0, mu_bc[:, 0:512])
                    _dump(1, r_bc[:, 0:512])
                    _dump(2, mu_bc[:, 512:1024])
                    _dump(3, r_bc[:, 512:1024])
                # apply LN1 + modulation -> xmT (bf16)
                xmT = [persist.tile([128, T], BF16, tag=f"xmT{k}", name="t39")
                       for k in range(8)]
                for k in range(8):
                    tb = tmp.tile([128, T], BF16, tag=f"lnt{k%2}", name="t40")
                    nc.vector.tensor_sub(tb, xT_sb[k], mu_bc)
                    nc.vector.tensor_mul(tb, tb, r_bc)
                    nc.scalar.activation(out=xmT[k], in_=tb, func=AF.Identity,
                                         scale=scaleA[:, k : k + 1],
                                         bias=biasA[:, k : k + 1])

                if dbg == "xmT":
                    for k in range(8):
                        _dump(k, xmT[k][:, 0:512])
                # ---- QKV projections
                wq = [wrow.tile([128, 1024], BF16, tag=f"wrow{k}", name="t41") for k in range(8)]
                for k in range(8):
                    nc.sync.dma_start(out=wq[k], in_=wqkv[128 * k : 128 * (k + 1), 0:1024])
                QT = [persist.tile([128, 512], BF16, tag=f"QT{g}", name="t42")
                      for g in range(8)]
                for g in range(8):
                    ps = ps_mm.tile([128, 512], F32, tag="mm", name="t43")
                    for k in range(8):
                        nc.tensor.matmul(ps, wq[k][:, 128 * g : 128 * (g + 1)],
                                         even_cols(xmT[k]),
                                         start=(k == 0), stop=(k == 7))
                    nc.vector.tensor_copy(out=QT[g], in_=ps)
                if dbg == "qt":
                    for g in range(8):
                        _dump(g, QT[g])
                wk = [wrow.tile([128, 1024], BF16, tag=f"wrow{k}", name="t44") for k in range(8)]
                for k in range(8):
                    nc.sync.dma_start(out=wk[k], in_=wqkv[128 * k : 128 * (k + 1), 1024:2048])
                KT = [persist.tile([128, T], BF16, tag=f"KT{g}", name="t45")
                      for g in range(8)]
                for g in range(8):
                    for half in range(2):
                        sl = slice(512 * half, 512 * (half + 1))
                        ps = ps_mm.tile([128, 512], F32, tag="mm", name="t46")
                        for k in range(8):
                            nc.tensor.matmul(ps, wk[k][:, 128 * g : 128 * (g + 1)],
                                             xmT[k][:, sl],
                                             start=(k == 0), stop=(k == 7))
                        nc.vector.tensor_copy(out=KT[g][:, sl], in_=ps)
                if dbg == "kt":
                    for g in range(8):
                        _dump(g, KT[g][:, 0:512])
                wv = [wrow.tile([128, 1024], BF16, tag=f"wrow{k}", name="t47") for k in range(8)]
                for k in range(8):
                    nc.sync.dma_start(out=wv[k], in_=wqkv[128 * k : 128 * (k + 1), 2048:3072])
                Vaug = [persist.tile([128, H, 65], BF16, tag=f"Vaug{tv}", name="t48")
                        for tv in range(8)]
                for tv in range(8):
                    nc.vector.memset(Vaug[tv][:, :, 64:65], 1.0)
                    for g in range(8):
                        ps = ps_s.tile([128, 128], F32, tag="ps_s", name="t49")
                        for k in range(8):
                            nc.tensor.matmul(ps, xmT[k][:, 128 * tv : 128 * (tv + 1)],
                                             wv[k][:, 128 * g : 128 * (g + 1)],
                                             start=(k == 0), stop=(k == 7))
                        nc.vector.tensor_copy(
                            out=Vaug[tv][:, 2 * g : 2 * g + 2, 0:64],
                            in_=ps.rearrange("p (a c) -> p a c", c=64))

                if dbg == "vaug":
                    for tv in range(8):
                        _dump(tv, Vaug[tv][:, 0:7, :], 455)
                # ---- attention (S^T scores, natural-O AV with l column)
                OT = [persist.tile([128, 512], BF16, tag=f"OT{k}", name="t50")
                      for k in range(8)]
                for i in range(NSLOT):
                    ej = 2 * i + 2
                    On = tmp.tile([128, 1024], BF16, tag=f"lnt{i % 2}", name="t55")
                    for h in range(H):
                        g, r0 = h // 2, 64 * (h % 2)
                        # one PSUM bank per head (pool rotates 2 banks), so the
                        # DVE reads of head h never share a bank with PE writes
                        # of head h+1
                        pso = ps_o.tile([128, 65], F32, tag="ps_o", name="t51")
                        # software pipeline: emit QK(j) before AV(j-1) so the
                        # PE never stalls on the ACT exp between them
                        prevE = None
                        prevV = None
                        for j in range(ej):
                            pss = ps_s.tile([128, 128], F32, tag="ps_s", name="t52")
                            nc.tensor.matmul(
                                pss, KT[g][r0 : r0 + 64, 128 * j : 128 * (j + 1)],
                                QT[g][r0 : r0 + 64, 128 * i : 128 * (i + 1)],
                                start=True, stop=True)
                            E = epool.tile([128, 128], BF16, tag="E", name="t53")
                            nc.scalar.activation(out=E, in_=pss, func=AF.Exp,
                                                 scale=ATT_SCALE, bias=nb20)
                            if j == 2 * i:
                                nc.vector.tensor_mul(E, E, mask_sb[i][0])
                            elif j == 2 * i + 1:
                                nc.vector.tensor_mul(E, E, mask_sb[i][1])
                            if prevE is not None:
                                nc.tensor.matmul(pso, prevE, prevV,
                                                 start=(j == 1), stop=False)
                            prevE, prevV = E, Vaug[j][:, h, :]
                        nc.tensor.matmul(pso, prevE, prevV,
                                         start=(ej == 1), stop=True)
                        rl1 = tmp.tile([128, 1], F32, tag=f"rl{h % 2}", name="t54")
                        nc.vector.reciprocal(rl1, pso[:, 64:65])
                        nc.vector.tensor_scalar_mul(
                            On[:, 64 * h : 64 * h + 64],
                            pso[:, 0:64], rl1)
                    for k in range(8):
                        pt = ps_tr.tile([128, 128], BF16, tag="tr", name="t56")
                        nc.tensor.transpose(pt, On[:, 128 * k : 128 * (k + 1)], id_sb)
                        nc.vector.tensor_copy(out=OT[k][:, 128 * i : 128 * (i + 1)],
                                              in_=pt)

                if dbg == "ot":
                    for k in range(8):
                        _dump(k, OT[k])
                # ---- attention out projection (transposed) + gated residual
                wp = [wrow.tile([128, 1024], BF16, tag=f"wrow{k}", name="t57") for k in range(8)]
                for k in range(8):
                    nc.sync.dma_start(out=wp[k], in_=wproj[128 * k : 128 * (k + 1), :])
                x2T = [persist.tile([128, 512], F32, tag=f"x2T{m}", name="t58")
                       for m in range(8)]
                for m in range(8):
                    ps = ps_mm.tile([128, 512], F32, tag="mm", name="t59")
                    for k in range(8):
                        nc.tensor.matmul(ps, wp[k][:, 128 * m : 128 * (m + 1)], OT[k],
                                         start=(k == 0), stop=(k == 7))
                    ta = tmp.tile([128, 512], F32, tag=f"res{m%2}", name="t60")
                    nc.scalar.activation(out=ta, in_=ps, func=AF.Identity,
                                         scale=modT[:, 16 + m : 17 + m],
                                         bias=gba[:, m : m + 1])
                    xo = tmp.tile([128, 512], F32, tag=f"xo{m % 2}", name="t61")
                    nc.vector.tensor_copy(out=xo, in_=even_cols(xT_sb[m]))
                    nc.vector.tensor_add(x2T[m], ta, xo)

                if dbg == "x2":
                    for m in range(8):
                        _dump(m, x2T[m])
                # ---- LN2 (own 512 tokens) + modulation -> xm2T
                x2b = [persist.tile([128, 512], BF16, tag=f"Vaug{m}", name="t62") for m in range(8)]
                for m in range(8):
                    nc.vector.tensor_copy(out=x2b[m], in_=x2T[m])
                x2sq = [tmp.tile([128, 512], BF16, tag=f"xsq{m % 2}", name="t63") for m in range(8)]
                for m in range(8):
                    nc.vector.tensor_mul(x2sq[m], x2b[m], x2b[m])
                mu2_ps = ps_mm.tile([128, 512], F32, tag="mm", name="t64")
                for m in range(8):
                    nc.tensor.matmul(mu2_ps[0:1, :], ones_col, x2b[m],
                                     start=(m == 0), stop=(m == 7))
                sums2 = tmp.tile([1, 1024], F32, tag="sums", name="t65")
                nc.vector.tensor_copy(out=sums2[:, 0:512], in_=mu2_ps[0:1, :])
                sq2_ps = ps_mm.tile([128, 512], F32, tag="mm", name="t66")
                for m in range(8):
                    nc.tensor.matmul(sq2_ps[0:1, :], ones_col, x2sq[m],
                                     start=(m == 0), stop=(m == 7))
                nc.vector.tensor_copy(out=sums2[:, 512:1024], in_=sq2_ps[0:1, :])
                mu2_row = tmp.tile([1, 512], F32, tag="mu_row", name="t67")
                nc.vector.tensor_scalar_mul(mu2_row, sums2[:, 0:512], 1.0 / D)
                msq2_row = tmp.tile([1, 512], F32, tag="msq_row", name="t68")
                nc.vector.tensor_scalar_mul(msq2_row, sums2[:, 512:1024], 1.0 / D)
                mu22 = tmp.tile([1, 512], F32, tag="sums", name="t69")
                nc.vector.tensor_mul(mu22, mu2_row, mu2_row)
                var2 = tmp.tile([1, 512], F32, tag="rowB", name="t70")
                nc.vector.tensor_sub(var2, msq2_row, mu22)
                std2 = tmp.tile([1, 512], F32, tag="msq_row", name="t71")
                nc.scalar.activation(out=std2, in_=var2, func=AF.Sqrt,
                                     bias=eps_sb, scale=1.0)
                r2_row = tmp.tile([1, 512], F32, tag="rowB2", name="t72")
                nc.vector.reciprocal(r2_row, std2)
                mu2_b = tmp.tile([1, 512], BF16, tag="mu_b", name="t73")
                nc.vector.tensor_copy(out=mu2_b, in_=mu2_row)
                r2_b = tmp.tile([1, 512], BF16, tag="r_b", name="t74")
                nc.vector.tensor_copy(out=r2_b, in_=r2_row)
                mu2_bc = persist.tile([128, 512], BF16, tag="mu_bc", name="t75")
                r2_bc = persist.tile([128, 512], BF16, tag="r_bc", name="t76")
                pb = ps_mm.tile([128, 512], F32, tag="mm", name="t77")
                nc.tensor.matmul(pb, ones_row, mu2_b, start=True, stop=True)
                nc.vector.tensor_copy(out=mu2_bc, in_=pb)
                pb2 = ps_mm.tile([128, 512], F32, tag="mm", name="t78")
                nc.tensor.matmul(pb2, ones_row, r2_b, start=True, stop=True)
                nc.vector.tensor_copy(out=r2_bc, in_=pb2)
                xm2T = [persist.tile([128, 512], BF16, tag=f"xmT{m}", name="t79")
                        for m in range(8)]
                for m in range(8):
                    tb = tmp.tile([128, 512], BF16, tag=f"lnt{m % 2}", name="t80")
                    nc.vector.tensor_sub(tb, x2b[m], mu2_bc)
                    nc.vector.tensor_mul(tb, tb, r2_bc)
                    nc.scalar.activation(out=xm2T[m], in_=tb, func=AF.Identity,
                                         scale=scaleM[:, m : m + 1],
                                         bias=biasM[:, m : m + 1])

                # ---- MLP mm1 + gelu -> hT
                def _ht_tag(hb):
                    if hb < 8:
                        return f"QT{hb}"
                    if hb < 16:
                        return f"OT{hb - 8}"
                    if hb < 24:
                        return f"xT{hb - 16}"
                    return f"KT{hb - 24}"
                hT = [persist.tile([128, 512], BF16, tag=_ht_tag(hb), name="t81")
                      for hb in range(32)]
                for hb in range(32):
                    wt = wstream.tile([128, 1024], BF16, tag="wada", name="t82")
                    nc.sync.dma_start(out=wt, in_=w1r[hb])
                    ps = ps_mm.tile([128, 512], F32, tag="mm", name="t83")
                    for k in range(8):
                        nc.tensor.matmul(ps, wt[:, 128 * k : 128 * (k + 1)], xm2T[k],
                                         start=(k == 0), stop=(k == 7))
                    nc.scalar.activation(out=hT[hb], in_=ps, func=AF.Gelu,
                                         bias=b1T_sb[:, hb : hb + 1], scale=1.0)

                # ---- MLP mm2 (transposed) + gated residual -> outT
                for m in range(8):
                    ps = ps_mm.tile([128, 512], F32, tag="mm", name="t85")
                    for hk in range(2):
                        w2t = w2s.tile([128, 2048], BF16, tag="w2t", name="t84")
                        nc.sync.dma_start(out=w2t, in_=w2r[m][:, 2048 * hk : 2048 * (hk + 1)])
                        for k in range(16):
                            nc.tensor.matmul(ps, w2t[:, 128 * k : 128 * (k + 1)],
                                             hT[16 * hk + k],
                                             start=(hk == 0 and k == 0),
                                             stop=(hk == 1 and k == 15))
                    tb = tmp.tile([128, 512], F32, tag=f"res{m % 2}", name="t86")
                    nc.scalar.activation(out=tb, in_=ps, func=AF.Identity,
                                         scale=modT[:, 40 + m : 41 + m],
                                         bias=gbm[:, m : m + 1])
                    to = tmp.tile([128, 512], F32, tag=f"xo{m % 2}", name="t87")
                    nc.vector.tensor_add(to, tb, x2T[m])
                    if not dbg:
                        nc.sync.dma_start(out=outT[128 * m : 128 * (m + 1), :], in_=to)
    _split_excess_waits(nc)
    return nc


# ------------------------------------------------------------------ host side
def _bf(a):
    return np.ascontiguousarray(a, dtype=np.float32).astype(ml_dtypes.bfloat16)


def host_prep(x, condition, Wqkv, Wproj, bproj, W1, b1, W2, b2, g1, be1, g2, be2,
              Wada, bada):
    x = np.asarray(x, np.float32)
    shared = {}
    shared["wqkv"] = _bf(Wqkv)
    shared["wproj"] = _bf(Wproj)
    shared["w1r"] = _bf(np.asarray(W1, np.float32).reshape(8, 128, 32, 128)
                        .transpose(2, 1, 0, 3).reshape(32, 128, 1024))
    shared["w2r"] = _bf(np.asarray(W2, np.float32).reshape(32, 128, 8, 128)
                        .transpose(2, 1, 0, 3).reshape(8, 128, 4096))
    bada_adj = np.asarray(bada, np.float32).copy()
    bada_adj[1024:2048] += 1.0   # (1 + sc_a)
    bada_adj[4096:5120] += 1.0   # (1 + sc_m)
    for nm, v in (("g1T", g1), ("be1T", be1), ("g2T", g2), ("be2T", be2),
                  ("bprojT", bproj), ("b2T", b2)):
        shared[nm] = np.ascontiguousarray(
            np.asarray(v, np.float32).reshape(8, 128).T)
    shared["b1T"] = np.ascontiguousarray(
        np.asarray(b1, np.float32).reshape(32, 128).T)
    shared["ident"] = _bf(np.eye(128, dtype=np.float32))

    cond = np.asarray(condition, np.float32)
    tri = np.triu(np.ones((128, 128), np.float32))  # [c, r] = c <= r
    Wada_f = np.asarray(Wada, np.float32)
    in_maps = []
    for b in range(4):
        silu_c = cond[b] / (1.0 + np.exp(-cond[b]))
        mod_full = silu_c @ Wada_f + bada_adj          # [6144] fp32
        modT_b = np.ascontiguousarray(mod_full.reshape(48, 128).T, np.float32)
        for p in range(2):
            perm = (np.arange(8) if p == 0
                    else np.array([1, 0, 3, 2, 5, 4, 7, 6]))
            xp = x[b].reshape(8, 128, D)[perm].reshape(T, D)
            m = np.zeros((NSLOT, 2, 128, 128), np.float32)
            for i in range(NSLOT):
                m[i, 0] = tri
                m[i, 1] = 0.0 if p == 0 else 1.0
            im = dict(shared)
            im["xT"] = _bf(xp.T)
            im["modT_d"] = modT_b
            im["masks"] = _bf(m)
            in_maps.append(im)
    return in_maps


_NC_CACHE = {}


def _get_nc(repeat=1):
    if repeat not in _NC_CACHE:
        _NC_CACHE[repeat] = build_nc(repeat)
    return _NC_CACHE[repeat]


def run_cores(in_maps, repeat=1):
    nc = _get_nc(repeat)
    res = run_bass_kernel_spmd(nc, in_maps, core_ids=list(range(8)))
    return [r["outT"] for r in res.results]


def kernel(**inputs):
    in_maps = host_prep(**inputs)
    outs = run_cores(in_maps)
    B = 4
    out = np.zeros((B, T, D), np.float32)
    for b in range(B):
        for p in range(2):
            o = outs[2 * b + p].T  # [512, 1024], slot i rows -> orig block 2i+p
            for i in range(NSLOT):
                out[b, 128 * (2 * i + p) : 128 * (2 * i + p + 1), :] = \
                    o[128 * i : 128 * (i + 1), :]
    return out

